# revision 15
# baseline (speedup 1.0000x reference)
"""DIEN-style GRU+AUGRU kernel for 8 Trainium2 NeuronCores (Bass/Tile).

Sharding (hardcoded): data-parallel over batch, B=4096 -> 512 rows/core,
no cross-core communication. Weights replicated; the item embedding table
is replicated in each core's DRAM and gathered on-device with indirect DMA.

On-chip layout: feature dim on partitions, batch (512) in free dim.
Single fused time loop: GRU step i, attention score i, AUGRU step i-K
(K-step software skew so attention sigmoids batch K steps per ACT op).
Gate pre-activations accumulate x-part + h-part in PSUM; biases ride the
ACT ops as per-partition vectors. z-gate weights are negated so sigmoid
yields w=1-z and both RNN updates share the form state + gate*(cand-state),
letting one packed [128,512] op chain update GRU (rows 0:64) and AUGRU
(rows 64:128) together.
"""

import sys

import numpy as np

if "/opt/trn_rl_repo" not in sys.path:
    sys.path.insert(0, "/opt/trn_rl_repo")

import concourse.bass as bass
import concourse.bacc as bacc
import concourse.mybir as mybir
import concourse.tile as tile
from concourse.bass import IndirectOffsetOnAxis
from concourse.masks import make_identity

AF = mybir.ActivationFunctionType
ALU = mybir.AluOpType
DT = mybir.dt

B, T, D = 4096, 200, 64
N_USERS, N_ITEMS = 100000, 100000
NCORES = 8
BL = B // NCORES          # 512 batch rows per core
NB = BL // 128            # 4 chunks of 128 tokens
K_LAG = 3                 # AUGRU lag == attention-sigmoid batch (PSUM 32-row slots at base 0/32/64)

F32 = DT.float32
F32R = DT.float32r
I32 = DT.int32


def build_nc(t_steps=T, hot_bf16=False):
    HOT = DT.float16 if hot_bf16 else F32
    WDT = HOT

    def m(ap):
        # f32r bitcast rejected by birverifier (inputs must be produced as
        # f32r); bf16 mode is the fast path, f32 mode is plain/slow.
        return ap

    nc = bacc.Bacc("TRN2", target_bir_lowering=False, debug=False)

    def transp(out, in_, identity):
        nc.tensor.transpose(out=out, in_=in_, identity=identity)

    # ---- DRAM I/O (per-core shapes) ----
    emb = nc.dram_tensor("emb", [N_ITEMS + 1, D], F32, kind="ExternalInput").ap()
    ubias = nc.dram_tensor("ubias", [N_USERS, 1], F32, kind="ExternalInput").ap()
    ibias = nc.dram_tensor("ibias", [N_ITEMS, 1], F32, kind="ExternalInput").ap()
    seqr = nc.dram_tensor("seqr", [128, NB * t_steps], I32, kind="ExternalInput").ap()
    uidx = nc.dram_tensor("uidx", [128, NB], I32, kind="ExternalInput").ap()
    iidx = nc.dram_tensor("iidx", [128, NB], I32, kind="ExternalInput").ap()

    wnames = ["w_rz_x", "w_rz_h", "w_arz_x", "w_arz_h"]
    wd = {n: nc.dram_tensor(n, [D, 128], WDT, kind="ExternalInput").ap() for n in wnames}
    for n in ["w_n_x", "w_n_h", "w_ah_x", "w_ah_h", "w_att"]:
        wd[n] = nc.dram_tensor(n, [D, D], WDT, kind="ExternalInput").ap()
    wd["w_aux"] = nc.dram_tensor("w_aux", [D, 1], WDT, kind="ExternalInput").ap()
    wd["ones_r"] = nc.dram_tensor("ones_r", [D, 32], WDT, kind="ExternalInput").ap()
    wd["ones32"] = nc.dram_tensor("ones32", [128, D], WDT, kind="ExternalInput").ap()
    bd = {}
    for n, p in [("b_rz", 128), ("b_arz", 128), ("b_tanh", 128),
                 ("b_hh_n", D), ("b_att", D), ("b_aux", 1)]:
        bd[n] = nc.dram_tensor(n, [p, 1], F32, kind="ExternalInput").ap()

    fm1_o = nc.dram_tensor("fm1_o", [BL, 1], F32, kind="ExternalOutput").ap()
    attn_o = nc.dram_tensor("attn_o", [BL, D], F32, kind="ExternalOutput").ap()
    aux_o = nc.dram_tensor("aux_o", [1, BL], F32, kind="ExternalOutput").ap()

    NG = (t_steps + K_LAG - 1) // K_LAG

    with tile.TileContext(nc) as tc:
        with (
            tc.tile_pool(name="const", bufs=1) as constp,
            tc.tile_pool(name="gat", bufs=4) as gatp,
            tc.tile_pool(name="etile", bufs=K_LAG + 3) as ep,
            tc.tile_pool(name="state", bufs=2) as statep,
            tc.tile_pool(name="work", bufs=3) as workp,
            tc.tile_pool(name="asb", bufs=2) as asbp,
            tc.tile_pool(name="psA", bufs=1, space="PSUM") as psA,
            tc.tile_pool(name="psB", bufs=1, space="PSUM") as psB,
            tc.tile_pool(name="psC", bufs=2, space="PSUM") as psC,
            tc.tile_pool(name="psT", bufs=2, space="PSUM") as psT,
            tc.tile_pool(name="psS", bufs=1, space="PSUM") as psS,
            tc.tile_pool(name="psR", bufs=1, space="PSUM") as psR,
        ):
            # ---- constants ----
            ident = constp.tile([128, 128], F32, tag="ident")
            make_identity(nc, ident[:])

            cw = {}
            for n, ap_ in wd.items():
                if n == "w_arz_h":
                    t_ = constp.tile([128, 128], WDT, tag=n)
                    nc.sync.dma_start(t_[64:128, :], ap_)
                else:
                    t_ = constp.tile(list(ap_.shape), WDT, tag=n)
                    nc.sync.dma_start(t_[:], ap_)
                cw[n] = t_
            cb = {}
            for n, ap_ in bd.items():
                if n == "b_hh_n":
                    # must share base partition (64) with the STT's SB input
                    t_ = constp.tile([128, 1], F32, tag=n)
                    nc.sync.dma_start(t_[64:128, :], ap_)
                else:
                    t_ = constp.tile(list(ap_.shape), F32, tag=n)
                    nc.sync.dma_start(t_[:], ap_)
                cb[n] = t_

            c_seqr = constp.tile([128, NB * t_steps], I32, tag="seqr")
            nc.sync.dma_start(c_seqr[:], seqr)
            c_uidx = constp.tile([128, NB], I32, tag="uidx")
            nc.sync.dma_start(c_uidx[:], uidx)
            c_iidx = constp.tile([128, NB], I32, tag="iidx")
            nc.sync.dma_start(c_iidx[:], iidx)

            # ---- fm1 ----  (HW indirect DMA honors one index per partition)
            g_ub = gatp.tile([128, NB], F32, tag="fm1a")
            for k in range(NB):
                nc.gpsimd.indirect_dma_start(
                    out=g_ub[:, k:k + 1], out_offset=None, in_=ubias,
                    in_offset=IndirectOffsetOnAxis(ap=c_uidx[:, k:k + 1], axis=0))
            g_ib = gatp.tile([128, NB], F32, tag="fm1b")
            for k in range(NB):
                nc.gpsimd.indirect_dma_start(
                    out=g_ib[:, k:k + 1], out_offset=None, in_=ibias,
                    in_offset=IndirectOffsetOnAxis(ap=c_iidx[:, k:k + 1], axis=0))
            fm1_sb = gatp.tile([128, NB], F32, tag="fm1c")
            # two-step chain: consecutive same-engine ops need no extra sem,
            # keeping each instruction within the ISA sync-wait slot limit
            nc.vector.tensor_copy(out=fm1_sb[:], in_=g_ub[:])
            nc.vector.tensor_tensor(out=fm1_sb[:], in0=fm1_sb[:], in1=g_ib[:],
                                    op=ALU.add)
            nc.sync.dma_start(fm1_o.rearrange("(k p) o -> p (k o)", p=128),
                              fm1_sb[:])

            # ---- target projection tp ----
            g_te = gatp.tile([128, NB * D], F32, tag="g")
            for k in range(NB):
                nc.gpsimd.indirect_dma_start(
                    out=g_te[:, k * D:(k + 1) * D], out_offset=None, in_=emb,
                    in_offset=IndirectOffsetOnAxis(ap=c_iidx[:, k:k + 1], axis=0))
            pp0 = psT.tile([128, BL], F32, tag="pp")
            for k in range(NB):
                transp(pp0[0:D, k * 128:(k + 1) * 128],
                       g_te[:, k * D:(k + 1) * D], ident[:])
            te_sb = ep.tile([D, BL], HOT, tag="e")
            nc.vector.tensor_copy(out=te_sb[:], in_=pp0[0:D, :])
            ptp = psB.tile([128, BL], F32, tag="p3")
            nc.tensor.matmul(out=ptp[0:D, :], lhsT=m(cw["w_att"][:]),
                             rhs=m(te_sb[:]), start=True, stop=True)
            tp_sb = constp.tile([D, BL], HOT, tag="tp")
            nc.vector.tensor_scalar_add(out=tp_sb[:], in0=ptp[0:D, :],
                                        scalar1=cb["b_att"][:])

            # ---- state ----
            hh_prev = statep.tile([128, BL], HOT, tag="hh")
            nc.gpsimd.memset(hh_prev[:], 0.0)

            e_tiles = [None] * t_steps
            a_groups = [None] * NG
            ps_s_cur = None

            # ---- fused recurrence ----
            for i in range(t_steps + K_LAG):
                gru_on = i < t_steps
                tau = i - K_LAG
                aug_on = tau >= 0

                hh_next = statep.tile([128, BL], HOT, tag="hh")
                tc_ps = psC.tile([128, BL], F32, tag="tc")
                rz = arz = prep = None

                if gru_on:
                    g = gatp.tile([128, NB * D], F32, tag="g")
                    for k in range(NB):
                        nc.gpsimd.indirect_dma_start(
                            out=g[:, k * D:(k + 1) * D], out_offset=None, in_=emb,
                            in_offset=IndirectOffsetOnAxis(
                                ap=c_seqr[:, i * NB + k:i * NB + k + 1], axis=0))
                    pp = psT.tile([128, BL], F32, tag="pp")
                    for k in range(NB):
                        transp(pp[0:D, k * 128:(k + 1) * 128],
                               g[:, k * D:(k + 1) * D], ident[:])
                    e = ep.tile([D, BL], HOT, tag="e")
                    nc.vector.tensor_copy(out=e[:], in_=pp[0:D, :])
                    e_tiles[i] = e

                    # GRU [w|r] pre-act, PSUM-accumulated x+h parts
                    p1 = psA.tile([128, BL], F32, tag="p1")
                    nc.tensor.matmul(out=p1[:], lhsT=m(cw["w_rz_x"][:]), rhs=m(e[:]),
                                     start=True, stop=False)
                    nc.tensor.matmul(out=p1[:], lhsT=m(cw["w_rz_h"][:]),
                                     rhs=m(hh_prev[0:D, :]), start=False, stop=True)
                    rz = workp.tile([128, BL], HOT, tag="rz")
                    nc.scalar.activation(out=rz[:], in_=p1[:], func=AF.Sigmoid,
                                         bias=cb["b_rz"][:])

                    # xn -> tc[0:64], ghn -> pp[64:128]
                    nc.tensor.matmul(out=tc_ps[0:D, :], lhsT=m(cw["w_n_x"][:]),
                                     rhs=m(e[:]), start=True, stop=True)
                    nc.tensor.matmul(out=pp[D:128, :], lhsT=m(cw["w_n_h"][:]),
                                     rhs=m(hh_prev[0:D, :]), start=True, stop=True)
                    t1 = workp.tile([D, BL], HOT, tag="t1")
                    nc.vector.scalar_tensor_tensor(
                        out=t1[:], in0=pp[D:128, :], scalar=cb["b_hh_n"][64:128, :],
                        in1=rz[D:128, :], op0=ALU.add, op1=ALU.mult)
                    nc.vector.tensor_tensor(out=tc_ps[0:D, :], in0=t1[:],
                                            in1=tc_ps[0:D, :], op=ALU.add)

                if aug_on:
                    e_tau = e_tiles[tau]
                    p3 = psB.tile([128, BL], F32, tag="p3")
                    nc.tensor.matmul(out=p3[:], lhsT=m(cw["w_arz_x"][:]),
                                     rhs=m(e_tau[:]), start=True, stop=False)
                    nc.tensor.matmul(out=p3[:], lhsT=m(cw["w_arz_h"][64:128, :]),
                                     rhs=m(hh_prev[D:128, :]), start=False, stop=True)
                    arz = workp.tile([128, BL], HOT, tag="arz")
                    nc.scalar.activation(out=arz[:], in_=p3[:], func=AF.Sigmoid,
                                         bias=cb["b_arz"][:])
                    # xh' -> tc[64:128]; += (r'*h') @ Wh'
                    nc.tensor.matmul(out=tc_ps[D:128, :], lhsT=m(cw["w_ah_x"][:]),
                                     rhs=m(e_tau[:]), start=True, stop=False)
                    rhp = workp.tile([D, BL], HOT, tag="rh")
                    nc.vector.tensor_tensor(out=rhp[:], in0=arz[D:128, :],
                                            in1=hh_prev[D:128, :], op=ALU.mult)
                    nc.tensor.matmul(out=tc_ps[D:128, :], lhsT=m(cw["w_ah_h"][:]),
                                     rhs=m(rhp[:]), start=False, stop=True)
                    # broadcast a_tau to 64 partitions: (1/32)*ones32.T @ a_rows
                    prep = psR.tile([D, BL], F32, tag="prep")
                    ga = a_groups[tau // K_LAG]
                    sl = (tau % K_LAG) * 32
                    nc.tensor.matmul(out=prep[:], lhsT=m(cw["ones32"][sl:sl + 32, :]),
                                     rhs=m(ga[sl:sl + 32, :]),
                                     start=True, stop=True)

                # ---- tanh + packed update ----
                if gru_on and aug_on:
                    nc.vector.tensor_tensor(out=rz[D:128, :], in0=prep[:],
                                            in1=arz[0:D, :], op=ALU.mult)
                    nh = workp.tile([128, BL], HOT, tag="nh")
                    nc.scalar.activation(out=nh[:], in_=tc_ps[:], func=AF.Tanh,
                                         bias=cb["b_tanh"][:])
                    s_t = workp.tile([128, BL], HOT, tag="s")
                    nc.vector.tensor_tensor(out=s_t[:], in0=nh[:], in1=hh_prev[:],
                                            op=ALU.subtract)
                    p_t = workp.tile([128, BL], HOT, tag="p")
                    nc.vector.tensor_tensor(out=p_t[:], in0=s_t[:], in1=rz[:],
                                            op=ALU.mult)
                    nc.vector.tensor_tensor(out=hh_next[:], in0=hh_prev[:],
                                            in1=p_t[:], op=ALU.add)
                elif gru_on:
                    nh = workp.tile([128, BL], HOT, tag="nh")
                    nc.scalar.activation(out=nh[0:D, :], in_=tc_ps[0:D, :],
                                         func=AF.Tanh, bias=cb["b_tanh"][0:D, :])
                    s_t = workp.tile([128, BL], HOT, tag="s")
                    nc.vector.tensor_tensor(out=s_t[0:D, :], in0=nh[0:D, :],
                                            in1=hh_prev[0:D, :], op=ALU.subtract)
                    p_t = workp.tile([128, BL], HOT, tag="p")
                    nc.vector.tensor_tensor(out=p_t[0:D, :], in0=s_t[0:D, :],
                                            in1=rz[0:D, :], op=ALU.mult)
                    nc.vector.tensor_tensor(out=hh_next[0:D, :],
                                            in0=hh_prev[0:D, :], in1=p_t[0:D, :],
                                            op=ALU.add)
                    nc.gpsimd.memset(hh_next[D:128, :], 0.0)
                else:
                    cz = workp.tile([128, BL], HOT, tag="rz")
                    nc.vector.tensor_tensor(out=cz[D:128, :], in0=prep[:],
                                            in1=arz[0:D, :], op=ALU.mult)
                    nh = workp.tile([128, BL], HOT, tag="nh")
                    nc.scalar.activation(out=nh[D:128, :], in_=tc_ps[D:128, :],
                                         func=AF.Tanh, bias=cb["b_tanh"][D:128, :])
                    s_t = workp.tile([128, BL], HOT, tag="s")
                    nc.vector.tensor_tensor(out=s_t[D:128, :], in0=nh[D:128, :],
                                            in1=hh_prev[D:128, :], op=ALU.subtract)
                    p_t = workp.tile([128, BL], HOT, tag="p")
                    nc.vector.tensor_tensor(out=p_t[D:128, :], in0=s_t[D:128, :],
                                            in1=cz[D:128, :], op=ALU.mult)
                    nc.vector.tensor_tensor(out=hh_next[D:128, :],
                                            in0=hh_prev[D:128, :],
                                            in1=p_t[D:128, :], op=ALU.add)
                    if i == t_steps + K_LAG - 1:
                        hf = statep.tile([D, BL], F32, tag="hf")
                        nc.vector.tensor_tensor(out=hf[:], in0=hh_prev[D:128, :],
                                                in1=p_t[D:128, :], op=ALU.add)

                if gru_on:
                    # attention score s_i = <h(i+1), tp> (post-update state)
                    am = workp.tile([D, BL], HOT, tag="am")
                    nc.vector.tensor_tensor(out=am[:], in0=tp_sb[:],
                                            in1=hh_next[0:D, :], op=ALU.mult)
                    gidx = i // K_LAG
                    slot = (i % K_LAG) * 32
                    if i % K_LAG == 0:
                        ps_s_cur = psS.tile([128, BL], F32, tag="ps_s")
                    nc.tensor.matmul(out=ps_s_cur[slot:slot + 32, :],
                                     lhsT=m(cw["ones_r"][:]), rhs=m(am[:]),
                                     start=True, stop=True)
                    if i % K_LAG == K_LAG - 1 or i == t_steps - 1:
                        nrow = (i % K_LAG + 1) * 32
                        a_sb = asbp.tile([96, BL], HOT, tag="a_sb")
                        nc.scalar.activation(out=a_sb[0:nrow, :],
                                             in_=ps_s_cur[0:nrow, :],
                                             func=AF.Sigmoid)
                        a_groups[gidx] = a_sb

                if i == t_steps - 1:
                    # aux logits from h(T)
                    ps_aux = psS.tile([128, BL], F32, tag="ps_s")
                    nc.tensor.matmul(out=ps_aux[0:1, :], lhsT=m(cw["w_aux"][:]),
                                     rhs=m(hh_next[0:D, :]), start=True, stop=True)
                    aux_sb = gatp.tile([1, BL], F32, tag="aux")
                    nc.vector.tensor_scalar_add(out=aux_sb[:], in0=ps_aux[0:1, :],
                                                scalar1=cb["b_aux"][:])
                    nc.sync.dma_start(aux_o, aux_sb[:])

                hh_prev = hh_next

            # ---- attn_vec output: h'(T) [64,512] -> [512,64] ----
            for k in range(NB):
                ppo = psT.tile([128, BL], F32, tag="pp")
                transp(ppo[:, 0:D], hf[:, k * 128:(k + 1) * 128],
                       ident[0:D, 0:D])
                av = gatp.tile([128, D], F32, tag="av")
                nc.vector.tensor_copy(out=av[:], in_=ppo[:, 0:D])
                nc.sync.dma_start(attn_o[k * 128:(k + 1) * 128, :], av[:])

    nc.compile()
    return nc


def host_prep(u_idx, i_idx, seq, item_emb, user_bias, item_bias,
              gru_w_ih, gru_b_ih, gru_w_hh, gru_b_hh, attn_w, attn_b,
              wr_w, wr_b, wz_w, wz_b, wh_w, wh_b, aux_w, aux_b,
              t_steps=T, hot_bf16=False):
    """Build per-core input maps (numpy only, cheap index/weight reshuffles)."""
    f32 = np.float32
    wdt = np.float16 if hot_bf16 else f32

    seq = np.asarray(seq).astype(np.int32)[:, :t_steps]
    u_idx = np.asarray(u_idx).astype(np.int32)
    i_idx = np.asarray(i_idx).astype(np.int32)
    item_emb = np.ascontiguousarray(np.asarray(item_emb, dtype=f32))
    user_bias = np.ascontiguousarray(np.asarray(user_bias, dtype=f32))
    item_bias = np.ascontiguousarray(np.asarray(item_bias, dtype=f32))

    wih = np.asarray(gru_w_ih, f32); bih = np.asarray(gru_b_ih, f32)
    whh = np.asarray(gru_w_hh, f32); bhh = np.asarray(gru_b_hh, f32)
    # torch GRU gate order r,z,n ; z negated so sigmoid gives w = 1-z
    w_rz_x = np.concatenate([-wih[D:2 * D].T, wih[0:D].T], axis=1)
    w_rz_h = np.concatenate([-whh[D:2 * D].T, whh[0:D].T], axis=1)
    b_rz = np.concatenate([-(bih[D:2 * D] + bhh[D:2 * D]),
                           bih[0:D] + bhh[0:D]])[:, None]
    w_n_x = wih[2 * D:3 * D].T.copy()
    w_n_h = whh[2 * D:3 * D].T.copy()
    b_hh_n = np.asarray(bhh[2 * D:3 * D], f32)[:, None]

    wrw = np.asarray(wr_w, f32); wzw = np.asarray(wz_w, f32)
    whw = np.asarray(wh_w, f32)
    # AUGRU rows: [z'; r']
    w_arz_x = np.concatenate([wzw[:, 0:D].T, wrw[:, 0:D].T], axis=1)
    w_arz_h = np.concatenate([wzw[:, D:2 * D].T, wrw[:, D:2 * D].T], axis=1)
    b_arz = np.concatenate([np.asarray(wz_b, f32), np.asarray(wr_b, f32)])[:, None]
    w_ah_x = whw[:, 0:D].T.copy()
    w_ah_h = whw[:, D:2 * D].T.copy()
    b_tanh = np.concatenate([bih[2 * D:3 * D], np.asarray(wh_b, f32)])[:, None]

    common = dict(
        emb=item_emb, ubias=user_bias, ibias=item_bias,
        w_rz_x=w_rz_x.astype(wdt), w_rz_h=w_rz_h.astype(wdt),
        w_n_x=w_n_x.astype(wdt), w_n_h=w_n_h.astype(wdt),
        w_arz_x=w_arz_x.astype(wdt), w_arz_h=w_arz_h.astype(wdt),
        w_ah_x=w_ah_x.astype(wdt), w_ah_h=w_ah_h.astype(wdt),
        w_att=np.asarray(attn_w, f32).T.copy().astype(wdt),
        w_aux=np.asarray(aux_w, f32).T.copy().astype(wdt),
        ones_r=np.ones((D, 32), wdt),
        ones32=np.full((128, D), 1.0 / 32.0, wdt),
        b_rz=b_rz, b_arz=b_arz, b_tanh=b_tanh, b_hh_n=b_hh_n,
        b_att=np.asarray(attn_b, f32)[:, None],
        b_aux=np.asarray(aux_b, f32).reshape(1, 1),
    )

    in_maps = []
    for c in range(NCORES):
        b0 = c * BL
        seqc = seq[b0:b0 + BL]
        seqr = seqc.reshape(NB, 128, t_steps).transpose(1, 2, 0).reshape(
            128, t_steps * NB)
        mm_ = dict(common)
        mm_.update(
            seqr=np.ascontiguousarray(seqr),
            uidx=np.ascontiguousarray(u_idx[b0:b0 + BL].reshape(NB, 128).T),
            iidx=np.ascontiguousarray(i_idx[b0:b0 + BL].reshape(NB, 128).T))
        in_maps.append(mm_)
    return in_maps


def assemble_outputs(results):
    fm1 = np.concatenate([r["fm1_o"] for r in results], axis=0)
    attn = np.concatenate([r["attn_o"] for r in results], axis=0)
    aux = np.concatenate([r["aux_o"][0] for r in results], axis=0)
    return fm1, attn, aux


_NC_CACHE = {}


def kernel(**inputs):
    from concourse.bass_utils import run_bass_kernel_spmd
    in_maps = host_prep(**inputs, hot_bf16=True)
    key = (T, True)
    if key not in _NC_CACHE:
        _NC_CACHE[key] = build_nc(t_steps=T, hot_bf16=True)
    nc = _NC_CACHE[key]
    res = run_bass_kernel_spmd(nc, in_maps, core_ids=list(range(NCORES)))
    return assemble_outputs(res.results)


# revision 17
# speedup vs baseline: 1.0001x; 1.0001x over previous
"""DIEN-style GRU+AUGRU kernel for 8 Trainium2 NeuronCores (Bass/Tile).

Sharding (hardcoded): data-parallel over batch, B=4096 -> 512 rows/core,
no cross-core communication. Weights replicated; the item embedding table
is replicated in each core's DRAM and gathered on-device with indirect DMA.

On-chip layout: feature dim on partitions, batch (512) in free dim.
Single fused time loop: GRU step i, attention score i, AUGRU step i-K
(K-step software skew so attention sigmoids batch K steps per ACT op).
Gate pre-activations accumulate x-part + h-part in PSUM; biases ride the
ACT ops as per-partition vectors. z-gate weights are negated so sigmoid
yields w=1-z and both RNN updates share the form state + gate*(cand-state),
letting one packed [128,512] op chain update GRU (rows 0:64) and AUGRU
(rows 64:128) together.
"""

import sys

import numpy as np

if "/opt/trn_rl_repo" not in sys.path:
    sys.path.insert(0, "/opt/trn_rl_repo")

import concourse.bass as bass
import concourse.bacc as bacc
import concourse.mybir as mybir
import concourse.tile as tile
from concourse.bass import IndirectOffsetOnAxis
from concourse.masks import make_identity

AF = mybir.ActivationFunctionType
ALU = mybir.AluOpType
DT = mybir.dt

B, T, D = 4096, 200, 64
N_USERS, N_ITEMS = 100000, 100000
NCORES = 8
BL = B // NCORES          # 512 batch rows per core
NB = BL // 128            # 4 chunks of 128 tokens
K_LAG = 3                 # AUGRU lag == attention-sigmoid batch (PSUM 32-row slots at base 0/32/64)

F32 = DT.float32
F32R = DT.float32r
I32 = DT.int32


def build_nc(t_steps=T, hot_bf16=False):
    HOT = DT.float16 if hot_bf16 else F32
    WDT = HOT

    def m(ap):
        # f32r bitcast rejected by birverifier (inputs must be produced as
        # f32r); bf16 mode is the fast path, f32 mode is plain/slow.
        return ap

    nc = bacc.Bacc("TRN2", target_bir_lowering=False, debug=False)

    def transp(out, in_, identity):
        nc.tensor.transpose(out=out, in_=in_, identity=identity)

    # ---- DRAM I/O (per-core shapes) ----
    emb = nc.dram_tensor("emb", [N_ITEMS + 1, D], F32, kind="ExternalInput").ap()
    ubias = nc.dram_tensor("ubias", [N_USERS, 1], F32, kind="ExternalInput").ap()
    ibias = nc.dram_tensor("ibias", [N_ITEMS, 1], F32, kind="ExternalInput").ap()
    seqr = nc.dram_tensor("seqr", [128, NB * t_steps], I32, kind="ExternalInput").ap()
    uidx = nc.dram_tensor("uidx", [128, NB], I32, kind="ExternalInput").ap()
    iidx = nc.dram_tensor("iidx", [128, NB], I32, kind="ExternalInput").ap()

    wnames = ["w_rz_x", "w_rz_h", "w_arz_x", "w_arz_h"]
    wd = {n: nc.dram_tensor(n, [D, 128], WDT, kind="ExternalInput").ap() for n in wnames}
    for n in ["w_n_x", "w_n_h", "w_ah_x", "w_ah_h", "w_att"]:
        wd[n] = nc.dram_tensor(n, [D, D], WDT, kind="ExternalInput").ap()
    wd["w_aux"] = nc.dram_tensor("w_aux", [D, 1], WDT, kind="ExternalInput").ap()
    wd["ones_r"] = nc.dram_tensor("ones_r", [D, 32], WDT, kind="ExternalInput").ap()
    wd["ones32"] = nc.dram_tensor("ones32", [128, D], WDT, kind="ExternalInput").ap()
    bd = {}
    for n, p in [("b_rz", 128), ("b_arz", 128), ("b_tanh", 128),
                 ("b_hh_n", D), ("b_att", D), ("b_aux", 1)]:
        bd[n] = nc.dram_tensor(n, [p, 1], F32, kind="ExternalInput").ap()

    fm1_o = nc.dram_tensor("fm1_o", [BL, 1], F32, kind="ExternalOutput").ap()
    attn_o = nc.dram_tensor("attn_o", [BL, D], F32, kind="ExternalOutput").ap()
    aux_o = nc.dram_tensor("aux_o", [1, BL], F32, kind="ExternalOutput").ap()

    NG = (t_steps + K_LAG - 1) // K_LAG

    with tile.TileContext(nc) as tc:
        with (
            tc.tile_pool(name="const", bufs=1) as constp,
            tc.tile_pool(name="gat", bufs=6) as gatp,
            tc.tile_pool(name="etile", bufs=K_LAG + 3) as ep,
            tc.tile_pool(name="state", bufs=2) as statep,
            tc.tile_pool(name="work", bufs=4) as workp,
            tc.tile_pool(name="asb", bufs=2) as asbp,
            tc.tile_pool(name="psA", bufs=1, space="PSUM") as psA,
            tc.tile_pool(name="psB", bufs=1, space="PSUM") as psB,
            tc.tile_pool(name="psC", bufs=2, space="PSUM") as psC,
            tc.tile_pool(name="psT", bufs=2, space="PSUM") as psT,
            tc.tile_pool(name="psS", bufs=1, space="PSUM") as psS,
            tc.tile_pool(name="psR", bufs=1, space="PSUM") as psR,
        ):
            # ---- constants ----
            ident = constp.tile([128, 128], F32, tag="ident")
            make_identity(nc, ident[:])

            cw = {}
            for n, ap_ in wd.items():
                if n == "w_arz_h":
                    t_ = constp.tile([128, 128], WDT, tag=n)
                    nc.sync.dma_start(t_[64:128, :], ap_)
                else:
                    t_ = constp.tile(list(ap_.shape), WDT, tag=n)
                    nc.sync.dma_start(t_[:], ap_)
                cw[n] = t_
            cb = {}
            for n, ap_ in bd.items():
                if n == "b_hh_n":
                    # must share base partition (64) with the STT's SB input
                    t_ = constp.tile([128, 1], F32, tag=n)
                    nc.sync.dma_start(t_[64:128, :], ap_)
                else:
                    t_ = constp.tile(list(ap_.shape), F32, tag=n)
                    nc.sync.dma_start(t_[:], ap_)
                cb[n] = t_

            c_seqr = constp.tile([128, NB * t_steps], I32, tag="seqr")
            nc.sync.dma_start(c_seqr[:], seqr)
            c_uidx = constp.tile([128, NB], I32, tag="uidx")
            nc.sync.dma_start(c_uidx[:], uidx)
            c_iidx = constp.tile([128, NB], I32, tag="iidx")
            nc.sync.dma_start(c_iidx[:], iidx)

            # ---- fm1 ----  (HW indirect DMA honors one index per partition)
            g_ub = gatp.tile([128, NB], F32, tag="fm1a")
            for k in range(NB):
                nc.gpsimd.indirect_dma_start(
                    out=g_ub[:, k:k + 1], out_offset=None, in_=ubias,
                    in_offset=IndirectOffsetOnAxis(ap=c_uidx[:, k:k + 1], axis=0))
            g_ib = gatp.tile([128, NB], F32, tag="fm1b")
            for k in range(NB):
                nc.gpsimd.indirect_dma_start(
                    out=g_ib[:, k:k + 1], out_offset=None, in_=ibias,
                    in_offset=IndirectOffsetOnAxis(ap=c_iidx[:, k:k + 1], axis=0))
            fm1_sb = gatp.tile([128, NB], F32, tag="fm1c")
            # two-step chain: consecutive same-engine ops need no extra sem,
            # keeping each instruction within the ISA sync-wait slot limit
            nc.vector.tensor_copy(out=fm1_sb[:], in_=g_ub[:])
            nc.vector.tensor_tensor(out=fm1_sb[:], in0=fm1_sb[:], in1=g_ib[:],
                                    op=ALU.add)
            nc.sync.dma_start(fm1_o.rearrange("(k p) o -> p (k o)", p=128),
                              fm1_sb[:])

            # ---- target projection tp ----
            g_te = gatp.tile([128, NB * D], F32, tag="g")
            for k in range(NB):
                nc.gpsimd.indirect_dma_start(
                    out=g_te[:, k * D:(k + 1) * D], out_offset=None, in_=emb,
                    in_offset=IndirectOffsetOnAxis(ap=c_iidx[:, k:k + 1], axis=0))
            pp0 = psT.tile([128, BL], F32, tag="pp")
            for k in range(NB):
                transp(pp0[0:D, k * 128:(k + 1) * 128],
                       g_te[:, k * D:(k + 1) * D], ident[:])
            te_sb = ep.tile([D, BL], HOT, tag="e")
            nc.vector.tensor_copy(out=te_sb[:], in_=pp0[0:D, :])
            ptp = psB.tile([128, BL], F32, tag="p3")
            nc.tensor.matmul(out=ptp[0:D, :], lhsT=m(cw["w_att"][:]),
                             rhs=m(te_sb[:]), start=True, stop=True)
            tp_sb = constp.tile([D, BL], HOT, tag="tp")
            nc.vector.tensor_scalar_add(out=tp_sb[:], in0=ptp[0:D, :],
                                        scalar1=cb["b_att"][:])

            # ---- state ----
            hh_prev = statep.tile([128, BL], HOT, tag="hh")
            nc.gpsimd.memset(hh_prev[:], 0.0)

            e_tiles = [None] * t_steps
            a_groups = [None] * NG
            ps_s_cur = None

            # ---- fused recurrence ----
            for i in range(t_steps + K_LAG):
                gru_on = i < t_steps
                tau = i - K_LAG
                aug_on = tau >= 0

                hh_next = statep.tile([128, BL], HOT, tag="hh")
                tc_ps = psC.tile([128, BL], F32, tag="tc")
                rz = arz = prep = None

                if gru_on:
                    g = gatp.tile([128, NB * D], F32, tag="g")
                    for k in range(NB):
                        nc.gpsimd.indirect_dma_start(
                            out=g[:, k * D:(k + 1) * D], out_offset=None, in_=emb,
                            in_offset=IndirectOffsetOnAxis(
                                ap=c_seqr[:, i * NB + k:i * NB + k + 1], axis=0))
                    pp = psT.tile([128, BL], F32, tag="pp")
                    for k in range(NB):
                        transp(pp[0:D, k * 128:(k + 1) * 128],
                               g[:, k * D:(k + 1) * D], ident[:])
                    e = ep.tile([D, BL], HOT, tag="e")
                    nc.vector.tensor_copy(out=e[:], in_=pp[0:D, :])
                    e_tiles[i] = e

                    # GRU [w|r] pre-act, PSUM-accumulated x+h parts
                    p1 = psA.tile([128, BL], F32, tag="p1")
                    nc.tensor.matmul(out=p1[:], lhsT=m(cw["w_rz_x"][:]), rhs=m(e[:]),
                                     start=True, stop=False)
                    nc.tensor.matmul(out=p1[:], lhsT=m(cw["w_rz_h"][:]),
                                     rhs=m(hh_prev[0:D, :]), start=False, stop=True)
                    rz = workp.tile([128, BL], HOT, tag="rz")
                    nc.scalar.activation(out=rz[:], in_=p1[:], func=AF.Sigmoid,
                                         bias=cb["b_rz"][:])

                    # xn -> tc[0:64], ghn -> pp[64:128]
                    nc.tensor.matmul(out=tc_ps[0:D, :], lhsT=m(cw["w_n_x"][:]),
                                     rhs=m(e[:]), start=True, stop=True)
                    nc.tensor.matmul(out=pp[D:128, :], lhsT=m(cw["w_n_h"][:]),
                                     rhs=m(hh_prev[0:D, :]), start=True, stop=True)
                    t1 = workp.tile([D, BL], HOT, tag="t1")
                    nc.vector.scalar_tensor_tensor(
                        out=t1[:], in0=pp[D:128, :], scalar=cb["b_hh_n"][64:128, :],
                        in1=rz[D:128, :], op0=ALU.add, op1=ALU.mult)
                    nc.vector.tensor_tensor(out=tc_ps[0:D, :], in0=t1[:],
                                            in1=tc_ps[0:D, :], op=ALU.add)

                if aug_on:
                    e_tau = e_tiles[tau]
                    p3 = psB.tile([128, BL], F32, tag="p3")
                    nc.tensor.matmul(out=p3[:], lhsT=m(cw["w_arz_x"][:]),
                                     rhs=m(e_tau[:]), start=True, stop=False)
                    nc.tensor.matmul(out=p3[:], lhsT=m(cw["w_arz_h"][64:128, :]),
                                     rhs=m(hh_prev[D:128, :]), start=False, stop=True)
                    arz = workp.tile([128, BL], HOT, tag="arz")
                    nc.scalar.activation(out=arz[:], in_=p3[:], func=AF.Sigmoid,
                                         bias=cb["b_arz"][:])
                    # xh' -> tc[64:128]; += (r'*h') @ Wh'
                    nc.tensor.matmul(out=tc_ps[D:128, :], lhsT=m(cw["w_ah_x"][:]),
                                     rhs=m(e_tau[:]), start=True, stop=False)
                    rhp = workp.tile([D, BL], HOT, tag="rh")
                    nc.vector.tensor_tensor(out=rhp[:], in0=arz[D:128, :],
                                            in1=hh_prev[D:128, :], op=ALU.mult)
                    nc.tensor.matmul(out=tc_ps[D:128, :], lhsT=m(cw["w_ah_h"][:]),
                                     rhs=m(rhp[:]), start=False, stop=True)
                    # broadcast a_tau to 64 partitions: (1/32)*ones32.T @ a_rows
                    prep = psR.tile([D, BL], F32, tag="prep")
                    ga = a_groups[tau // K_LAG]
                    sl = (tau % K_LAG) * 32
                    nc.tensor.matmul(out=prep[:], lhsT=m(cw["ones32"][sl:sl + 32, :]),
                                     rhs=m(ga[sl:sl + 32, :]),
                                     start=True, stop=True)

                # ---- tanh + packed update ----
                if gru_on and aug_on:
                    nc.vector.tensor_tensor(out=rz[D:128, :], in0=prep[:],
                                            in1=arz[0:D, :], op=ALU.mult)
                    nh = workp.tile([128, BL], HOT, tag="nh")
                    nc.scalar.activation(out=nh[:], in_=tc_ps[:], func=AF.Tanh,
                                         bias=cb["b_tanh"][:])
                    s_t = workp.tile([128, BL], HOT, tag="s")
                    nc.vector.tensor_tensor(out=s_t[:], in0=nh[:], in1=hh_prev[:],
                                            op=ALU.subtract)
                    p_t = workp.tile([128, BL], HOT, tag="p")
                    nc.vector.tensor_tensor(out=p_t[:], in0=s_t[:], in1=rz[:],
                                            op=ALU.mult)
                    nc.vector.tensor_tensor(out=hh_next[:], in0=hh_prev[:],
                                            in1=p_t[:], op=ALU.add)
                elif gru_on:
                    nh = workp.tile([128, BL], HOT, tag="nh")
                    nc.scalar.activation(out=nh[0:D, :], in_=tc_ps[0:D, :],
                                         func=AF.Tanh, bias=cb["b_tanh"][0:D, :])
                    s_t = workp.tile([128, BL], HOT, tag="s")
                    nc.vector.tensor_tensor(out=s_t[0:D, :], in0=nh[0:D, :],
                                            in1=hh_prev[0:D, :], op=ALU.subtract)
                    p_t = workp.tile([128, BL], HOT, tag="p")
                    nc.vector.tensor_tensor(out=p_t[0:D, :], in0=s_t[0:D, :],
                                            in1=rz[0:D, :], op=ALU.mult)
                    nc.vector.tensor_tensor(out=hh_next[0:D, :],
                                            in0=hh_prev[0:D, :], in1=p_t[0:D, :],
                                            op=ALU.add)
                    nc.gpsimd.memset(hh_next[D:128, :], 0.0)
                else:
                    cz = workp.tile([128, BL], HOT, tag="rz")
                    nc.vector.tensor_tensor(out=cz[D:128, :], in0=prep[:],
                                            in1=arz[0:D, :], op=ALU.mult)
                    nh = workp.tile([128, BL], HOT, tag="nh")
                    nc.scalar.activation(out=nh[D:128, :], in_=tc_ps[D:128, :],
                                         func=AF.Tanh, bias=cb["b_tanh"][D:128, :])
                    s_t = workp.tile([128, BL], HOT, tag="s")
                    nc.vector.tensor_tensor(out=s_t[D:128, :], in0=nh[D:128, :],
                                            in1=hh_prev[D:128, :], op=ALU.subtract)
                    p_t = workp.tile([128, BL], HOT, tag="p")
                    nc.vector.tensor_tensor(out=p_t[D:128, :], in0=s_t[D:128, :],
                                            in1=cz[D:128, :], op=ALU.mult)
                    nc.vector.tensor_tensor(out=hh_next[D:128, :],
                                            in0=hh_prev[D:128, :],
                                            in1=p_t[D:128, :], op=ALU.add)
                    if i == t_steps + K_LAG - 1:
                        hf = statep.tile([D, BL], F32, tag="hf")
                        nc.vector.tensor_tensor(out=hf[:], in0=hh_prev[D:128, :],
                                                in1=p_t[D:128, :], op=ALU.add)

                if gru_on:
                    # attention score s_i = <h(i+1), tp> (post-update state)
                    am = workp.tile([D, BL], HOT, tag="am")
                    nc.vector.tensor_tensor(out=am[:], in0=tp_sb[:],
                                            in1=hh_next[0:D, :], op=ALU.mult)
                    gidx = i // K_LAG
                    slot = (i % K_LAG) * 32
                    if i % K_LAG == 0:
                        ps_s_cur = psS.tile([128, BL], F32, tag="ps_s")
                    nc.tensor.matmul(out=ps_s_cur[slot:slot + 32, :],
                                     lhsT=m(cw["ones_r"][:]), rhs=m(am[:]),
                                     start=True, stop=True)
                    if i % K_LAG == K_LAG - 1 or i == t_steps - 1:
                        nrow = (i % K_LAG + 1) * 32
                        a_sb = asbp.tile([96, BL], HOT, tag="a_sb")
                        nc.scalar.activation(out=a_sb[0:nrow, :],
                                             in_=ps_s_cur[0:nrow, :],
                                             func=AF.Sigmoid)
                        a_groups[gidx] = a_sb

                if i == t_steps - 1:
                    # aux logits from h(T)
                    ps_aux = psS.tile([128, BL], F32, tag="ps_s")
                    nc.tensor.matmul(out=ps_aux[0:1, :], lhsT=m(cw["w_aux"][:]),
                                     rhs=m(hh_next[0:D, :]), start=True, stop=True)
                    aux_sb = gatp.tile([1, BL], F32, tag="aux")
                    nc.vector.tensor_scalar_add(out=aux_sb[:], in0=ps_aux[0:1, :],
                                                scalar1=cb["b_aux"][:])
                    nc.sync.dma_start(aux_o, aux_sb[:])

                hh_prev = hh_next

            # ---- attn_vec output: h'(T) [64,512] -> [512,64] ----
            for k in range(NB):
                ppo = psT.tile([128, BL], F32, tag="pp")
                transp(ppo[:, 0:D], hf[:, k * 128:(k + 1) * 128],
                       ident[0:D, 0:D])
                av = gatp.tile([128, D], F32, tag="av")
                nc.vector.tensor_copy(out=av[:], in_=ppo[:, 0:D])
                nc.sync.dma_start(attn_o[k * 128:(k + 1) * 128, :], av[:])

    nc.compile()
    return nc


def host_prep(u_idx, i_idx, seq, item_emb, user_bias, item_bias,
              gru_w_ih, gru_b_ih, gru_w_hh, gru_b_hh, attn_w, attn_b,
              wr_w, wr_b, wz_w, wz_b, wh_w, wh_b, aux_w, aux_b,
              t_steps=T, hot_bf16=False):
    """Build per-core input maps (numpy only, cheap index/weight reshuffles)."""
    f32 = np.float32
    wdt = np.float16 if hot_bf16 else f32

    seq = np.asarray(seq).astype(np.int32)[:, :t_steps]
    u_idx = np.asarray(u_idx).astype(np.int32)
    i_idx = np.asarray(i_idx).astype(np.int32)
    item_emb = np.ascontiguousarray(np.asarray(item_emb, dtype=f32))
    user_bias = np.ascontiguousarray(np.asarray(user_bias, dtype=f32))
    item_bias = np.ascontiguousarray(np.asarray(item_bias, dtype=f32))

    wih = np.asarray(gru_w_ih, f32); bih = np.asarray(gru_b_ih, f32)
    whh = np.asarray(gru_w_hh, f32); bhh = np.asarray(gru_b_hh, f32)
    # torch GRU gate order r,z,n ; z negated so sigmoid gives w = 1-z
    w_rz_x = np.concatenate([-wih[D:2 * D].T, wih[0:D].T], axis=1)
    w_rz_h = np.concatenate([-whh[D:2 * D].T, whh[0:D].T], axis=1)
    b_rz = np.concatenate([-(bih[D:2 * D] + bhh[D:2 * D]),
                           bih[0:D] + bhh[0:D]])[:, None]
    w_n_x = wih[2 * D:3 * D].T.copy()
    w_n_h = whh[2 * D:3 * D].T.copy()
    b_hh_n = np.asarray(bhh[2 * D:3 * D], f32)[:, None]

    wrw = np.asarray(wr_w, f32); wzw = np.asarray(wz_w, f32)
    whw = np.asarray(wh_w, f32)
    # AUGRU rows: [z'; r']
    w_arz_x = np.concatenate([wzw[:, 0:D].T, wrw[:, 0:D].T], axis=1)
    w_arz_h = np.concatenate([wzw[:, D:2 * D].T, wrw[:, D:2 * D].T], axis=1)
    b_arz = np.concatenate([np.asarray(wz_b, f32), np.asarray(wr_b, f32)])[:, None]
    w_ah_x = whw[:, 0:D].T.copy()
    w_ah_h = whw[:, D:2 * D].T.copy()
    b_tanh = np.concatenate([bih[2 * D:3 * D], np.asarray(wh_b, f32)])[:, None]

    common = dict(
        emb=item_emb, ubias=user_bias, ibias=item_bias,
        w_rz_x=w_rz_x.astype(wdt), w_rz_h=w_rz_h.astype(wdt),
        w_n_x=w_n_x.astype(wdt), w_n_h=w_n_h.astype(wdt),
        w_arz_x=w_arz_x.astype(wdt), w_arz_h=w_arz_h.astype(wdt),
        w_ah_x=w_ah_x.astype(wdt), w_ah_h=w_ah_h.astype(wdt),
        w_att=np.asarray(attn_w, f32).T.copy().astype(wdt),
        w_aux=np.asarray(aux_w, f32).T.copy().astype(wdt),
        ones_r=np.ones((D, 32), wdt),
        ones32=np.full((128, D), 1.0 / 32.0, wdt),
        b_rz=b_rz, b_arz=b_arz, b_tanh=b_tanh, b_hh_n=b_hh_n,
        b_att=np.asarray(attn_b, f32)[:, None],
        b_aux=np.asarray(aux_b, f32).reshape(1, 1),
    )

    in_maps = []
    for c in range(NCORES):
        b0 = c * BL
        seqc = seq[b0:b0 + BL]
        seqr = seqc.reshape(NB, 128, t_steps).transpose(1, 2, 0).reshape(
            128, t_steps * NB)
        mm_ = dict(common)
        mm_.update(
            seqr=np.ascontiguousarray(seqr),
            uidx=np.ascontiguousarray(u_idx[b0:b0 + BL].reshape(NB, 128).T),
            iidx=np.ascontiguousarray(i_idx[b0:b0 + BL].reshape(NB, 128).T))
        in_maps.append(mm_)
    return in_maps


def assemble_outputs(results):
    fm1 = np.concatenate([r["fm1_o"] for r in results], axis=0)
    attn = np.concatenate([r["attn_o"] for r in results], axis=0)
    aux = np.concatenate([r["aux_o"][0] for r in results], axis=0)
    return fm1, attn, aux


_NC_CACHE = {}


def kernel(**inputs):
    from concourse.bass_utils import run_bass_kernel_spmd
    in_maps = host_prep(**inputs, hot_bf16=True)
    key = (T, True)
    if key not in _NC_CACHE:
        _NC_CACHE[key] = build_nc(t_steps=T, hot_bf16=True)
    nc = _NC_CACHE[key]
    res = run_bass_kernel_spmd(nc, in_maps, core_ids=list(range(NCORES)))
    return assemble_outputs(res.results)
